# revision 6
# baseline (speedup 1.0000x reference)
"""Trainium2 Bass kernel for nn_AttentionMechanism (batched attention with
per-sample queries), data-parallel across 8 NeuronCores.

Math (per batch row b):
    q = msgs @ Wq.T + bq                         [H]
    k_t = Wk @ tau_t + bk ; scores_t = q.k_t/32
    alpha = softmax(scores) ; out = sum_t alpha_t (Wv @ tau_t + bv)

Rewrite used here (exact up to softmax shift invariance):
    qk   = (msgs @ Wq.T + bq) @ Wk  = msgs @ (Wq.T @ Wk) + bq @ Wk   [TAU]
    scores_t = qk . tau_t / 32      (the q.bk term is constant in t -> cancels)
    p_t  = exp(scores_t)            (scores are O(1), no max-subtraction needed)
    ctx  = sum_t p_t tau_t / sum_t p_t
    out  = ctx @ Wv.T + bv          (uses sum alpha = 1)

This removes the [B,T,H] k-projection (17 GFLOP/core) and [B,T,VDIM]
v-projection entirely; the kernel streams tau once from HBM (32 MB/core).
"""

import math

import numpy as np

import concourse.bass as bass
import concourse.bacc as bacc
import concourse.tile as tile
from concourse import mybir
from concourse.bass_utils import run_bass_kernel_spmd
from concourse.masks import make_identity

F32 = mybir.dt.float32
BF16 = mybir.dt.bfloat16

B = 2048
T = 32
TAU = 1024
MSG = 512
HID = 1024
VDIM = 128
N_CORES = 8
B_LOCAL = B // N_CORES

Alu = mybir.AluOpType
Act = mybir.ActivationFunctionType


def build(b_local=B_LOCAL, t_chunk=4, chunk_bufs=4, prod_bufs=2,
          cast_engine="gpsimd", sum_engine="scalar"):
    assert b_local % 128 == 0 and T % t_chunk == 0
    n_btiles = b_local // 128
    n_chunks = T // t_chunk
    chunk_free = t_chunk * TAU

    nc = bacc.Bacc("TRN2", target_bir_lowering=False, debug=False)

    traj = nc.declare_dram_parameter(
        "imagined_trajectory", [b_local, T * TAU], F32, isOutput=False
    )
    msgs = nc.declare_dram_parameter(
        "received_messages", [b_local, MSG], F32, isOutput=False
    )
    Wq = nc.declare_dram_parameter("Wq", [HID, MSG], F32, isOutput=False)
    bq = nc.declare_dram_parameter("bq", [HID], F32, isOutput=False)
    Wk = nc.declare_dram_parameter("Wk", [HID, TAU], F32, isOutput=False)
    Wv = nc.declare_dram_parameter("Wv", [VDIM, TAU], F32, isOutput=False)
    bv = nc.declare_dram_parameter("bv", [VDIM], F32, isOutput=False)
    out = nc.declare_dram_parameter("out", [b_local, VDIM], F32, isOutput=True)

    HQ = HID // 128  # 8 h-chunks
    MQ = MSG // 128  # 4 m-chunks
    CQ = TAU // 128  # 8 c-chunks

    with tile.TileContext(nc) as tc:
        with (
            tc.tile_pool(name="const", bufs=1) as const,
            tc.tile_pool(name="persist", bufs=1) as persist,
            tc.tile_pool(name="psum", bufs=2, space="PSUM") as psum,
            tc.tile_pool(name="psum_tr", bufs=2, space="PSUM") as psum_tr,
        ):
            ident_f = const.tile([128, 128], F32)
            make_identity(nc, ident_f)
            onespad_b = const.tile([128, 128], BF16)
            nc.vector.memset(onespad_b, 0.0)
            nc.vector.memset(onespad_b[0:1, :], 1.0)
            bv_sb = const.tile([1, VDIM], F32)
            nc.sync.dma_start(out=bv_sb, in_=bv[None, :])
            bvpad_b = const.tile([128, VDIM], BF16)
            nc.vector.memset(bvpad_b, 0.0)
            nc.vector.tensor_copy(out=bvpad_b[0:1, :], in_=bv_sb)
            WvT_b = persist.tile([128, CQ, VDIM], BF16)  # [c-part, c-chunk, d]
            qk_sb = [
                persist.tile([128, TAU], F32, tag=f"qk{i}", name=f"qk{i}")
                for i in range(n_btiles)
            ]
            qk_b = [
                persist.tile([128, TAU], BF16, tag=f"qkb{i}", name=f"qkb{i}")
                for i in range(n_btiles)
            ]

            # ---------- setup: weights, fused projection, per-sample qk ----------
            with tc.tile_pool(name="wtmp", bufs=1) as wtmp:
                Wq_sb = wtmp.tile([128, HQ, MSG], F32)  # [h-part, h-chunk, m]
                nc.sync.dma_start(
                    out=Wq_sb, in_=Wq[:, :].rearrange("(j p) m -> p j m", p=128)
                )
                Wk_sb = wtmp.tile([128, HQ, TAU], F32)  # [h-part, h-chunk, c]
                nc.sync.dma_start(
                    out=Wk_sb, in_=Wk[:, :].rearrange("(j p) c -> p j c", p=128)
                )
                Wq_b = wtmp.tile([128, HQ, MSG], BF16)
                nc.vector.tensor_copy(out=Wq_b, in_=Wq_sb)
                Wk_b = wtmp.tile([128, HQ, TAU], BF16)
                nc.vector.tensor_copy(out=Wk_b, in_=Wk_sb)

                # bq -> [h-part, h-chunk]
                bq_sb = wtmp.tile([128, HQ], F32)
                nc.sync.dma_start(
                    out=bq_sb, in_=bq[:].rearrange("(j p) -> p j", p=128)
                )

                # Wv [VDIM=128, TAU] -> WvT blocks [c-part, d]
                Wv_sb = wtmp.tile([VDIM, TAU], F32)
                nc.sync.dma_start(out=Wv_sb, in_=Wv[:, :])
                for j in range(CQ):
                    pt = psum_tr.tile([128, 128], F32, tag="tr", name="pt")
                    nc.tensor.transpose(pt, Wv_sb[:, j * 128 : (j + 1) * 128], ident_f)
                    nc.scalar.copy(out=WvT_b[:, j, :], in_=pt)

                # msgs -> msgsT blocks [m-part, b]
                msgsT_b = wtmp.tile([128, MQ, b_local], BF16)
                for bi in range(n_btiles):
                    ms = wtmp.tile([128, MSG], F32, tag="msgs_f32")
                    nc.sync.dma_start(
                        out=ms, in_=msgs[bi * 128 : (bi + 1) * 128, :]
                    )
                    for mi in range(MQ):
                        pt = psum_tr.tile([128, 128], F32, tag="tr", name="pt")
                        nc.tensor.transpose(
                            pt, ms[:, mi * 128 : (mi + 1) * 128], ident_f
                        )
                        nc.scalar.copy(
                            out=msgsT_b[:, mi, bi * 128 : (bi + 1) * 128], in_=pt
                        )

                # Wfused[m, c] = sum_h Wq[h, m] * Wk[h, c]   (= Wq.T @ Wk)
                Wfused_b = wtmp.tile([128, MQ, TAU], BF16)
                for mi in range(MQ):
                    pf = psum.tile([128, TAU], F32, tag="mm")
                    for nh in range(2):
                        nsl = slice(nh * 512, (nh + 1) * 512)
                        for j in range(HQ):
                            nc.tensor.matmul(
                                pf[:, nsl],
                                lhsT=Wq_b[:, j, mi * 128 : (mi + 1) * 128],
                                rhs=Wk_b[:, j, nsl],
                                start=(j == 0),
                                stop=(j == HQ - 1),
                            )
                    nc.scalar.copy(out=Wfused_b[:, mi, :], in_=pf)

                # qk_bias[c] = sum_h bq[h] * Wk[h, c]
                pb = psum.tile([1, TAU], F32, tag="mm", name="pb")
                for nh in range(2):
                    nsl = slice(nh * 512, (nh + 1) * 512)
                    for j in range(HQ):
                        nc.tensor.matmul(
                            pb[:, nsl],
                            lhsT=bq_sb[:, j : j + 1],
                            rhs=Wk_sb[:, j, nsl],
                            start=(j == 0),
                            stop=(j == HQ - 1),
                        )
                wf_bias_b = wtmp.tile([128, TAU], BF16)
                nc.vector.memset(wf_bias_b, 0.0)
                nc.scalar.copy(out=wf_bias_b[0:1, :], in_=pb)
                ones_row_b = wtmp.tile([128, b_local], BF16)
                nc.vector.memset(ones_row_b, 0.0)
                nc.vector.memset(ones_row_b[0:1, :], 1.0)

                # qk[b, c] = msgs @ Wfused + qk_bias, scaled by 1/sqrt(H)
                for bi in range(n_btiles):
                    pq = psum.tile([128, TAU], F32, tag="mm")
                    for nh in range(2):
                        nsl = slice(nh * 512, (nh + 1) * 512)
                        for mi in range(MQ):
                            nc.tensor.matmul(
                                pq[:, nsl],
                                lhsT=msgsT_b[:, mi, bi * 128 : (bi + 1) * 128],
                                rhs=Wfused_b[:, mi, nsl],
                                start=(mi == 0),
                                stop=False,
                            )
                        nc.tensor.matmul(
                            pq[:, nsl],
                            lhsT=ones_row_b[:, bi * 128 : (bi + 1) * 128],
                            rhs=wf_bias_b[:, nsl],
                            start=False,
                            stop=True,
                        )
                    nc.scalar.mul(out=qk_sb[bi], in_=pq, mul=1.0 / math.sqrt(HID))
                    nc.vector.tensor_copy(out=qk_b[bi], in_=qk_sb[bi])

            # ---------- main loop: stream tau, scores -> exp -> weighted sum ----
            with (
                tc.tile_pool(name="stream", bufs=chunk_bufs) as stream,
                tc.tile_pool(name="bfp", bufs=prod_bufs) as bfp,
                tc.tile_pool(name="aux", bufs=2) as aux,
                tc.tile_pool(name="outp", bufs=2) as outp,
            ):
                dump = aux.tile([128, TAU], BF16, tag="dump", name="dump", bufs=1)
                for bi in range(n_btiles):
                    bsl = slice(bi * 128, (bi + 1) * 128)
                    ctx_b = aux.tile([128, TAU], BF16, tag="ctx", name="ctx_b")
                    nc.vector.memset(ctx_b, 0.0)
                    scores = aux.tile([128, T], F32, tag="scores", name="scores")
                    p_t = aux.tile([128, T], F32, tag="p", name="p_t")

                    for ci in range(n_chunks):
                        chunk = stream.tile([128, chunk_free], F32, tag="chunk", name="chunk")
                        c0 = ci * chunk_free
                        nc.sync.dma_start(
                            out=chunk, in_=traj[bsl, c0 : c0 + chunk_free]
                        )
                        chunk_bf = bfp.tile([128, chunk_free], BF16, tag="cbf", name="chunk_bf")
                        if cast_engine == "gpsimd":
                            nc.gpsimd.tensor_copy(out=chunk_bf, in_=chunk)
                        elif cast_engine == "scalar":
                            nc.scalar.copy(out=chunk_bf, in_=chunk)
                        else:
                            nc.vector.tensor_copy(out=chunk_bf, in_=chunk)
                        prod = bfp.tile([128, chunk_free], BF16, tag="prod", name="prod")
                        for tt in range(t_chunk):
                            col = ci * t_chunk + tt
                            seg = slice(tt * TAU, (tt + 1) * TAU)
                            nc.vector.tensor_tensor(
                                out=prod[:, seg],
                                in0=chunk_bf[:, seg],
                                in1=qk_b[bi],
                                op=Alu.mult,
                            )
                            if sum_engine == "scalar":
                                nc.scalar.activation(
                                    out=dump,
                                    in_=prod[:, seg],
                                    func=Act.Copy,
                                    accum_out=scores[:, col : col + 1],
                                )
                            else:
                                nc.vector.tensor_reduce(
                                    out=scores[:, col : col + 1],
                                    in_=prod[:, seg],
                                    axis=mybir.AxisListType.X,
                                    op=Alu.add,
                                )
                        csl = slice(ci * t_chunk, (ci + 1) * t_chunk)
                        nc.scalar.activation(
                            out=p_t[:, csl], in_=scores[:, csl], func=Act.Exp
                        )
                        for tt in range(t_chunk):
                            col = ci * t_chunk + tt
                            seg = slice(tt * TAU, (tt + 1) * TAU)
                            nc.vector.tensor_scalar(
                                out=prod[:, seg],
                                in0=chunk_bf[:, seg],
                                scalar1=p_t[:, col : col + 1],
                                scalar2=None,
                                op0=Alu.mult,
                            )
                            nc.vector.tensor_tensor(
                                out=ctx_b,
                                in0=ctx_b,
                                in1=prod[:, seg],
                                op=Alu.add,
                            )

                    # normalize, project: out = (ctx / sum p) @ Wv.T + bv
                    s_sum = aux.tile([128, 1], F32, tag="ssum", name="s_sum")
                    nc.vector.tensor_reduce(
                        out=s_sum, in_=p_t, axis=mybir.AxisListType.X, op=Alu.add
                    )
                    rinv = aux.tile([128, 1], F32, tag="rinv", name="rinv")
                    nc.vector.reciprocal(out=rinv, in_=s_sum)
                    ctxn_f = aux.tile([128, TAU], F32, tag="ctxn", name="ctxn_f")
                    nc.vector.tensor_scalar(
                        out=ctxn_f,
                        in0=ctx_b,
                        scalar1=rinv,
                        scalar2=None,
                        op0=Alu.mult,
                    )
                    ctxT_b = aux.tile([128, CQ, 128], BF16, tag="ctxT", name="ctxT_b")
                    for j in range(CQ):
                        ptb = psum_tr.tile([128, 128], F32, tag="tr", name="ptb")
                        nc.tensor.transpose(
                            ptb, ctxn_f[:, j * 128 : (j + 1) * 128], ident_f
                        )
                        nc.scalar.copy(out=ctxT_b[:, j, :], in_=ptb)
                    pm = psum.tile([128, VDIM], F32, tag="mm", name="pm")
                    for j in range(CQ):
                        nc.tensor.matmul(
                            pm,
                            lhsT=ctxT_b[:, j, :],
                            rhs=WvT_b[:, j, :],
                            start=(j == 0),
                            stop=False,
                        )
                    nc.tensor.matmul(
                        pm,
                        lhsT=onespad_b,
                        rhs=bvpad_b,
                        start=False,
                        stop=True,
                    )
                    msg_out = outp.tile([128, VDIM], F32, tag="msg", name="msg_out")
                    nc.scalar.copy(out=msg_out, in_=pm)
                    nc.sync.dma_start(out=out[bsl, :], in_=msg_out)

    nc.compile()
    return nc


_NC_CACHE = {}


def _get_nc():
    key = "default"
    if key not in _NC_CACHE:
        _NC_CACHE[key] = build()
    return _NC_CACHE[key]


def make_in_maps(imagined_trajectory, received_messages, Wq, bq, Wk, Wv, bv):
    bl = B_LOCAL
    in_maps = []
    for i in range(N_CORES):
        sl = slice(i * bl, (i + 1) * bl)
        in_maps.append(
            {
                "imagined_trajectory": np.ascontiguousarray(
                    imagined_trajectory[sl], dtype=np.float32
                ),
                "received_messages": np.ascontiguousarray(
                    received_messages[sl], dtype=np.float32
                ),
                "Wq": np.asarray(Wq, dtype=np.float32),
                "bq": np.asarray(bq, dtype=np.float32),
                "Wk": np.asarray(Wk, dtype=np.float32),
                "Wv": np.asarray(Wv, dtype=np.float32),
                "bv": np.asarray(bv, dtype=np.float32),
            }
        )
    return in_maps


def kernel(
    imagined_trajectory,
    received_messages,
    Wq,
    bq,
    Wk,
    bk,
    Wv,
    bv,
):
    nc = _get_nc()
    in_maps = make_in_maps(
        imagined_trajectory, received_messages, Wq, bq, Wk, Wv, bv
    )
    res = run_bass_kernel_spmd(nc, in_maps, list(range(N_CORES)))
    return np.concatenate([res.results[i]["out"] for i in range(N_CORES)], axis=0)


# revision 7
# speedup vs baseline: 1.7016x; 1.7016x over previous
"""Trainium2 Bass kernel for nn_AttentionMechanism (batched attention with
per-sample queries), data-parallel across 8 NeuronCores.

Math (per batch row b):
    q = msgs @ Wq.T + bq                         [H]
    k_t = Wk @ tau_t + bk ; scores_t = q.k_t/32
    alpha = softmax(scores) ; out = sum_t alpha_t (Wv @ tau_t + bv)

Rewrite used here (exact up to softmax shift invariance):
    qk   = (msgs @ Wq.T + bq) @ Wk  = msgs @ (Wq.T @ Wk) + bq @ Wk   [TAU]
    scores_t = qk . tau_t / 32      (the q.bk term is constant in t -> cancels)
    p_t  = exp(scores_t)            (scores are O(1), no max-subtraction needed)
    ctx  = sum_t p_t tau_t / sum_t p_t
    out  = ctx @ Wv.T + bv          (uses sum alpha = 1)

This removes the [B,T,H] k-projection (17 GFLOP/core) and [B,T,VDIM]
v-projection entirely; the kernel streams tau once from HBM (32 MB/core).
"""

import math

import numpy as np

import concourse.bass as bass
import concourse.bacc as bacc
import concourse.tile as tile
from concourse import mybir
from concourse.bass_utils import run_bass_kernel_spmd
from concourse.masks import make_identity

F32 = mybir.dt.float32
BF16 = mybir.dt.bfloat16

B = 2048
T = 32
TAU = 1024
MSG = 512
HID = 1024
VDIM = 128
N_CORES = 8
B_LOCAL = B // N_CORES

Alu = mybir.AluOpType
Act = mybir.ActivationFunctionType


def build(b_local=B_LOCAL, t_chunk=4, chunk_bufs=4, prod_bufs=2,
          cast_dve_every=3, sum_engine="scalar", bcast_mult=True):
    assert b_local % 128 == 0 and T % t_chunk == 0
    n_btiles = b_local // 128
    n_chunks = T // t_chunk
    chunk_free = t_chunk * TAU

    nc = bacc.Bacc("TRN2", target_bir_lowering=False, debug=False)

    traj = nc.declare_dram_parameter(
        "imagined_trajectory", [b_local, T * TAU], F32, isOutput=False
    )
    msgs = nc.declare_dram_parameter(
        "received_messages", [b_local, MSG], F32, isOutput=False
    )
    Wq = nc.declare_dram_parameter("Wq", [HID, MSG], F32, isOutput=False)
    bq = nc.declare_dram_parameter("bq", [HID], F32, isOutput=False)
    Wk = nc.declare_dram_parameter("Wk", [HID, TAU], F32, isOutput=False)
    Wv = nc.declare_dram_parameter("Wv", [VDIM, TAU], F32, isOutput=False)
    bv = nc.declare_dram_parameter("bv", [VDIM], F32, isOutput=False)
    out = nc.declare_dram_parameter("out", [b_local, VDIM], F32, isOutput=True)

    HQ = HID // 128  # 8 h-chunks
    MQ = MSG // 128  # 4 m-chunks
    CQ = TAU // 128  # 8 c-chunks

    with tile.TileContext(nc) as tc:
        with (
            tc.tile_pool(name="const", bufs=1) as const,
            tc.tile_pool(name="persist", bufs=1) as persist,
            tc.tile_pool(name="psum", bufs=2, space="PSUM") as psum,
            tc.tile_pool(name="psum_tr", bufs=2, space="PSUM") as psum_tr,
        ):
            ident_f = const.tile([128, 128], F32)
            make_identity(nc, ident_f)
            onespad_b = const.tile([128, 128], BF16)
            nc.vector.memset(onespad_b, 0.0)
            nc.vector.memset(onespad_b[0:1, :], 1.0)
            bv_sb = const.tile([1, VDIM], F32)
            nc.sync.dma_start(out=bv_sb, in_=bv[None, :])
            bvpad_b = const.tile([128, VDIM], BF16)
            nc.vector.memset(bvpad_b, 0.0)
            nc.vector.tensor_copy(out=bvpad_b[0:1, :], in_=bv_sb)
            WvT_b = persist.tile([128, CQ, VDIM], BF16)  # [c-part, c-chunk, d]
            qk_sb = [
                persist.tile([128, TAU], F32, tag=f"qk{i}", name=f"qk{i}")
                for i in range(n_btiles)
            ]
            qk_b = [
                persist.tile([128, TAU], BF16, tag=f"qkb{i}", name=f"qkb{i}")
                for i in range(n_btiles)
            ]

            # ---------- setup: weights, fused projection, per-sample qk ----------
            with tc.tile_pool(name="wtmp", bufs=1) as wtmp:
                Wq_sb = wtmp.tile([128, HQ, MSG], F32)  # [h-part, h-chunk, m]
                nc.sync.dma_start(
                    out=Wq_sb, in_=Wq[:, :].rearrange("(j p) m -> p j m", p=128)
                )
                Wk_sb = wtmp.tile([128, HQ, TAU], F32)  # [h-part, h-chunk, c]
                nc.sync.dma_start(
                    out=Wk_sb, in_=Wk[:, :].rearrange("(j p) c -> p j c", p=128)
                )
                Wq_b = wtmp.tile([128, HQ, MSG], BF16)
                nc.vector.tensor_copy(out=Wq_b, in_=Wq_sb)
                Wk_b = wtmp.tile([128, HQ, TAU], BF16)
                nc.vector.tensor_copy(out=Wk_b, in_=Wk_sb)

                # bq -> [h-part, h-chunk]
                bq_sb = wtmp.tile([128, HQ], F32)
                nc.sync.dma_start(
                    out=bq_sb, in_=bq[:].rearrange("(j p) -> p j", p=128)
                )

                # Wv [VDIM=128, TAU] -> WvT blocks [c-part, d]
                Wv_sb = wtmp.tile([VDIM, TAU], F32)
                nc.sync.dma_start(out=Wv_sb, in_=Wv[:, :])
                for j in range(CQ):
                    pt = psum_tr.tile([128, 128], F32, tag="tr", name="pt")
                    nc.tensor.transpose(pt, Wv_sb[:, j * 128 : (j + 1) * 128], ident_f)
                    nc.scalar.copy(out=WvT_b[:, j, :], in_=pt)

                # msgs -> msgsT blocks [m-part, b]
                msgsT_b = wtmp.tile([128, MQ, b_local], BF16)
                for bi in range(n_btiles):
                    ms = wtmp.tile([128, MSG], F32, tag="msgs_f32")
                    nc.sync.dma_start(
                        out=ms, in_=msgs[bi * 128 : (bi + 1) * 128, :]
                    )
                    for mi in range(MQ):
                        pt = psum_tr.tile([128, 128], F32, tag="tr", name="pt")
                        nc.tensor.transpose(
                            pt, ms[:, mi * 128 : (mi + 1) * 128], ident_f
                        )
                        nc.scalar.copy(
                            out=msgsT_b[:, mi, bi * 128 : (bi + 1) * 128], in_=pt
                        )

                # Wfused[m, c] = sum_h Wq[h, m] * Wk[h, c]   (= Wq.T @ Wk)
                Wfused_b = wtmp.tile([128, MQ, TAU], BF16)
                for mi in range(MQ):
                    pf = psum.tile([128, TAU], F32, tag="mm")
                    for nh in range(2):
                        nsl = slice(nh * 512, (nh + 1) * 512)
                        for j in range(HQ):
                            nc.tensor.matmul(
                                pf[:, nsl],
                                lhsT=Wq_b[:, j, mi * 128 : (mi + 1) * 128],
                                rhs=Wk_b[:, j, nsl],
                                start=(j == 0),
                                stop=(j == HQ - 1),
                            )
                    nc.scalar.copy(out=Wfused_b[:, mi, :], in_=pf)

                # qk_bias[c] = sum_h bq[h] * Wk[h, c]
                pb = psum.tile([1, TAU], F32, tag="mm", name="pb")
                for nh in range(2):
                    nsl = slice(nh * 512, (nh + 1) * 512)
                    for j in range(HQ):
                        nc.tensor.matmul(
                            pb[:, nsl],
                            lhsT=bq_sb[:, j : j + 1],
                            rhs=Wk_sb[:, j, nsl],
                            start=(j == 0),
                            stop=(j == HQ - 1),
                        )
                wf_bias_b = wtmp.tile([128, TAU], BF16)
                nc.vector.memset(wf_bias_b, 0.0)
                nc.scalar.copy(out=wf_bias_b[0:1, :], in_=pb)
                ones_row_b = wtmp.tile([128, b_local], BF16)
                nc.vector.memset(ones_row_b, 0.0)
                nc.vector.memset(ones_row_b[0:1, :], 1.0)

                # qk[b, c] = msgs @ Wfused + qk_bias, scaled by 1/sqrt(H)
                for bi in range(n_btiles):
                    pq = psum.tile([128, TAU], F32, tag="mm")
                    for nh in range(2):
                        nsl = slice(nh * 512, (nh + 1) * 512)
                        for mi in range(MQ):
                            nc.tensor.matmul(
                                pq[:, nsl],
                                lhsT=msgsT_b[:, mi, bi * 128 : (bi + 1) * 128],
                                rhs=Wfused_b[:, mi, nsl],
                                start=(mi == 0),
                                stop=False,
                            )
                        nc.tensor.matmul(
                            pq[:, nsl],
                            lhsT=ones_row_b[:, bi * 128 : (bi + 1) * 128],
                            rhs=wf_bias_b[:, nsl],
                            start=False,
                            stop=True,
                        )
                    nc.scalar.mul(out=qk_sb[bi], in_=pq, mul=1.0 / math.sqrt(HID))
                    nc.vector.tensor_copy(out=qk_b[bi], in_=qk_sb[bi])

            # ---------- main loop: stream tau, scores -> exp -> weighted sum ----
            with (
                tc.tile_pool(name="stream", bufs=chunk_bufs) as stream,
                tc.tile_pool(name="bfp", bufs=prod_bufs) as bfp,
                tc.tile_pool(name="aux", bufs=2) as aux,
                tc.tile_pool(name="outp", bufs=2) as outp,
            ):
                dump = aux.tile([128, TAU], BF16, tag="dump", name="dump", bufs=1)
                for bi in range(n_btiles):
                    bsl = slice(bi * 128, (bi + 1) * 128)
                    ctx_pp = [
                        aux.tile([128, TAU], BF16, tag="ctxA", name="ctxA"),
                        aux.tile([128, TAU], BF16, tag="ctxB", name="ctxB"),
                    ]
                    pp = 0
                    nc.vector.memset(ctx_pp[0], 0.0)
                    scores = aux.tile([128, T], F32, tag="scores", name="scores")
                    p_t = aux.tile([128, T], F32, tag="p", name="p_t")

                    for ci in range(n_chunks):
                        chunk = stream.tile([128, chunk_free], F32, tag="chunk", name="chunk")
                        c0 = ci * chunk_free
                        nc.sync.dma_start(
                            out=chunk, in_=traj[bsl, c0 : c0 + chunk_free]
                        )
                        chunk_bf = bfp.tile([128, chunk_free], BF16, tag="cbf", name="chunk_bf")
                        if cast_dve_every and ci % cast_dve_every == 0:
                            nc.vector.tensor_copy(out=chunk_bf, in_=chunk)
                        else:
                            nc.scalar.copy(out=chunk_bf, in_=chunk)
                        prod = bfp.tile([128, chunk_free], BF16, tag="prod", name="prod")
                        if bcast_mult:
                            qk_rep = bass.AP(
                                tensor=qk_b[bi].tensor,
                                offset=qk_b[bi].offset,
                                ap=[qk_b[bi].ap[0], [0, t_chunk], [1, TAU]],
                            )
                            nc.vector.tensor_tensor(
                                out=prod,
                                in0=chunk_bf,
                                in1=qk_rep,
                                op=Alu.mult,
                            )
                        else:
                            for tt in range(t_chunk):
                                seg = slice(tt * TAU, (tt + 1) * TAU)
                                nc.vector.tensor_tensor(
                                    out=prod[:, seg],
                                    in0=chunk_bf[:, seg],
                                    in1=qk_b[bi],
                                    op=Alu.mult,
                                )
                        for tt in range(t_chunk):
                            col = ci * t_chunk + tt
                            seg = slice(tt * TAU, (tt + 1) * TAU)
                            if sum_engine == "scalar":
                                nc.scalar.activation(
                                    out=dump,
                                    in_=prod[:, seg],
                                    func=Act.Copy,
                                    accum_out=scores[:, col : col + 1],
                                )
                            else:
                                nc.vector.tensor_reduce(
                                    out=scores[:, col : col + 1],
                                    in_=prod[:, seg],
                                    axis=mybir.AxisListType.X,
                                    op=Alu.add,
                                )
                        csl = slice(ci * t_chunk, (ci + 1) * t_chunk)
                        nc.scalar.activation(
                            out=p_t[:, csl], in_=scores[:, csl], func=Act.Exp
                        )
                        for tt in range(t_chunk):
                            col = ci * t_chunk + tt
                            seg = slice(tt * TAU, (tt + 1) * TAU)
                            nc.vector.tensor_scalar(
                                out=prod[:, seg],
                                in0=chunk_bf[:, seg],
                                scalar1=p_t[:, col : col + 1],
                                scalar2=None,
                                op0=Alu.mult,
                            )
                            src_ctx, dst_ctx = ctx_pp[pp], ctx_pp[1 - pp]
                            pp = 1 - pp
                            nc.vector.tensor_tensor(
                                out=dst_ctx,
                                in0=src_ctx,
                                in1=prod[:, seg],
                                op=Alu.add,
                            )

                    # normalize, project: out = (ctx / sum p) @ Wv.T + bv
                    s_sum = aux.tile([128, 1], F32, tag="ssum", name="s_sum")
                    nc.vector.tensor_reduce(
                        out=s_sum, in_=p_t, axis=mybir.AxisListType.X, op=Alu.add
                    )
                    rinv = aux.tile([128, 1], F32, tag="rinv", name="rinv")
                    nc.vector.reciprocal(out=rinv, in_=s_sum)
                    ctxn_f = aux.tile([128, TAU], F32, tag="ctxn", name="ctxn_f")
                    nc.vector.tensor_scalar(
                        out=ctxn_f,
                        in0=ctx_pp[pp],
                        scalar1=rinv,
                        scalar2=None,
                        op0=Alu.mult,
                    )
                    ctxT_b = aux.tile([128, CQ, 128], BF16, tag="ctxT", name="ctxT_b")
                    for j in range(CQ):
                        ptb = psum_tr.tile([128, 128], F32, tag="tr", name="ptb")
                        nc.tensor.transpose(
                            ptb, ctxn_f[:, j * 128 : (j + 1) * 128], ident_f
                        )
                        nc.scalar.copy(out=ctxT_b[:, j, :], in_=ptb)
                    pm = psum.tile([128, VDIM], F32, tag="mm", name="pm")
                    for j in range(CQ):
                        nc.tensor.matmul(
                            pm,
                            lhsT=ctxT_b[:, j, :],
                            rhs=WvT_b[:, j, :],
                            start=(j == 0),
                            stop=False,
                        )
                    nc.tensor.matmul(
                        pm,
                        lhsT=onespad_b,
                        rhs=bvpad_b,
                        start=False,
                        stop=True,
                    )
                    msg_out = outp.tile([128, VDIM], F32, tag="msg", name="msg_out")
                    nc.scalar.copy(out=msg_out, in_=pm)
                    nc.sync.dma_start(out=out[bsl, :], in_=msg_out)

    nc.compile()
    return nc


_NC_CACHE = {}


def _get_nc():
    key = "default"
    if key not in _NC_CACHE:
        _NC_CACHE[key] = build()
    return _NC_CACHE[key]


def make_in_maps(imagined_trajectory, received_messages, Wq, bq, Wk, Wv, bv):
    bl = B_LOCAL
    in_maps = []
    for i in range(N_CORES):
        sl = slice(i * bl, (i + 1) * bl)
        in_maps.append(
            {
                "imagined_trajectory": np.ascontiguousarray(
                    imagined_trajectory[sl], dtype=np.float32
                ),
                "received_messages": np.ascontiguousarray(
                    received_messages[sl], dtype=np.float32
                ),
                "Wq": np.asarray(Wq, dtype=np.float32),
                "bq": np.asarray(bq, dtype=np.float32),
                "Wk": np.asarray(Wk, dtype=np.float32),
                "Wv": np.asarray(Wv, dtype=np.float32),
                "bv": np.asarray(bv, dtype=np.float32),
            }
        )
    return in_maps


def kernel(
    imagined_trajectory,
    received_messages,
    Wq,
    bq,
    Wk,
    bk,
    Wv,
    bv,
):
    nc = _get_nc()
    in_maps = make_in_maps(
        imagined_trajectory, received_messages, Wq, bq, Wk, Wv, bv
    )
    res = run_bass_kernel_spmd(nc, in_maps, list(range(N_CORES)))
    return np.concatenate([res.results[i]["out"] for i in range(N_CORES)], axis=0)


# revision 9
# speedup vs baseline: 1.7477x; 1.0271x over previous
"""Trainium2 Bass kernel for nn_AttentionMechanism (batched attention with
per-sample queries), data-parallel across 8 NeuronCores.

Math (per batch row b):
    q = msgs @ Wq.T + bq                         [H]
    k_t = Wk @ tau_t + bk ; scores_t = q.k_t/32
    alpha = softmax(scores) ; out = sum_t alpha_t (Wv @ tau_t + bv)

Rewrite used here (exact up to softmax shift invariance):
    qk   = (msgs @ Wq.T + bq) @ Wk  = msgs @ (Wq.T @ Wk) + bq @ Wk   [TAU]
    scores_t = qk . tau_t / 32      (the q.bk term is constant in t -> cancels)
    p_t  = exp(scores_t)            (scores are O(1), no max-subtraction needed)
    ctx  = sum_t p_t tau_t / sum_t p_t
    out  = ctx @ Wv.T + bv          (uses sum alpha = 1)

This removes the [B,T,H] k-projection (17 GFLOP/core) and [B,T,VDIM]
v-projection entirely; the kernel streams tau once from HBM (32 MB/core).
"""

import math

import numpy as np

import concourse.bass as bass
import concourse.bacc as bacc
import concourse.tile as tile
from concourse import mybir
from concourse.bass_utils import run_bass_kernel_spmd
from concourse.masks import make_identity

F32 = mybir.dt.float32
BF16 = mybir.dt.bfloat16

B = 2048
T = 32
TAU = 1024
MSG = 512
HID = 1024
VDIM = 128
N_CORES = 8
B_LOCAL = B // N_CORES

Alu = mybir.AluOpType
Act = mybir.ActivationFunctionType


def build(b_local=B_LOCAL, t_chunk=8, chunk_bufs=4, prod_bufs=2,
          ts_act_per_chunk=2, sum_engine="scalar", dma_cast=True,
          bcast_mult=True):
    assert b_local % 128 == 0 and T % t_chunk == 0
    n_btiles = b_local // 128
    n_chunks = T // t_chunk
    chunk_free = t_chunk * TAU

    nc = bacc.Bacc("TRN2", target_bir_lowering=False, debug=False)

    traj = nc.declare_dram_parameter(
        "imagined_trajectory", [b_local, T * TAU], F32, isOutput=False
    )
    msgs = nc.declare_dram_parameter(
        "received_messages", [b_local, MSG], F32, isOutput=False
    )
    Wq = nc.declare_dram_parameter("Wq", [HID, MSG], F32, isOutput=False)
    bq = nc.declare_dram_parameter("bq", [HID], F32, isOutput=False)
    Wk = nc.declare_dram_parameter("Wk", [HID, TAU], F32, isOutput=False)
    Wv = nc.declare_dram_parameter("Wv", [VDIM, TAU], F32, isOutput=False)
    bv = nc.declare_dram_parameter("bv", [VDIM], F32, isOutput=False)
    out = nc.declare_dram_parameter("out", [b_local, VDIM], F32, isOutput=True)

    HQ = HID // 128  # 8 h-chunks
    MQ = MSG // 128  # 4 m-chunks
    CQ = TAU // 128  # 8 c-chunks

    with tile.TileContext(nc) as tc:
        with (
            tc.tile_pool(name="const", bufs=1) as const,
            tc.tile_pool(name="persist", bufs=1) as persist,
            tc.tile_pool(name="psum", bufs=2, space="PSUM") as psum,
            tc.tile_pool(name="psum_tr", bufs=2, space="PSUM") as psum_tr,
        ):
            ident_f = const.tile([128, 128], F32)
            make_identity(nc, ident_f)
            onespad_b = const.tile([128, 128], BF16)
            nc.vector.memset(onespad_b, 0.0)
            nc.vector.memset(onespad_b[0:1, :], 1.0)
            bv_sb = const.tile([1, VDIM], F32)
            nc.sync.dma_start(out=bv_sb, in_=bv[None, :])
            bvpad_b = const.tile([128, VDIM], BF16)
            nc.vector.memset(bvpad_b, 0.0)
            nc.vector.tensor_copy(out=bvpad_b[0:1, :], in_=bv_sb)
            WvT_b = persist.tile([128, CQ, VDIM], BF16)  # [c-part, c-chunk, d]
            qk_b = [
                persist.tile([128, TAU], BF16, tag=f"qkb{i}", name=f"qkb{i}")
                for i in range(n_btiles)
            ]

            # ---------- setup: weights, fused projection, per-sample qk ----------
            with tc.tile_pool(name="wtmp", bufs=1) as wtmp:
                Wq_b = wtmp.tile([128, HQ, MSG], BF16)  # [h-part, h-chunk, m]
                nc.gpsimd.dma_start(
                    out=Wq_b, in_=Wq[:, :].rearrange("(j p) m -> p j m", p=128)
                )
                Wk_b = wtmp.tile([128, HQ, TAU], BF16)  # [h-part, h-chunk, c]
                nc.gpsimd.dma_start(
                    out=Wk_b, in_=Wk[:, :].rearrange("(j p) c -> p j c", p=128)
                )

                # bq -> [h-part, h-chunk]
                bq_b = wtmp.tile([128, HQ], BF16)
                nc.gpsimd.dma_start(
                    out=bq_b, in_=bq[:].rearrange("(j p) -> p j", p=128)
                )

                # Wv [VDIM=128, TAU] -> WvT blocks [c-part, d]
                Wv_sb = wtmp.tile([VDIM, TAU], F32)
                nc.sync.dma_start(out=Wv_sb, in_=Wv[:, :])
                for j in range(CQ):
                    pt = psum_tr.tile([128, 128], F32, tag="tr", name="pt")
                    nc.tensor.transpose(pt, Wv_sb[:, j * 128 : (j + 1) * 128], ident_f)
                    nc.scalar.copy(out=WvT_b[:, j, :], in_=pt)

                # msgs -> msgsT blocks [m-part, b]
                msgsT_b = wtmp.tile([128, MQ, b_local], BF16)
                for bi in range(n_btiles):
                    ms = wtmp.tile([128, MSG], F32, tag="msgs_f32")
                    nc.sync.dma_start(
                        out=ms, in_=msgs[bi * 128 : (bi + 1) * 128, :]
                    )
                    for mi in range(MQ):
                        pt = psum_tr.tile([128, 128], F32, tag="tr", name="pt")
                        nc.tensor.transpose(
                            pt, ms[:, mi * 128 : (mi + 1) * 128], ident_f
                        )
                        nc.scalar.copy(
                            out=msgsT_b[:, mi, bi * 128 : (bi + 1) * 128], in_=pt
                        )

                # Wfused[m, c] = sum_h Wq[h, m] * Wk[h, c]   (= Wq.T @ Wk)
                Wfused_b = wtmp.tile([128, MQ, TAU], BF16)
                for mi in range(MQ):
                    pf = psum.tile([128, TAU], F32, tag="mm")
                    for nh in range(2):
                        nsl = slice(nh * 512, (nh + 1) * 512)
                        for j in range(HQ):
                            nc.tensor.matmul(
                                pf[:, nsl],
                                lhsT=Wq_b[:, j, mi * 128 : (mi + 1) * 128],
                                rhs=Wk_b[:, j, nsl],
                                start=(j == 0),
                                stop=(j == HQ - 1),
                            )
                    nc.scalar.copy(out=Wfused_b[:, mi, :], in_=pf)

                # qk_bias[c] = sum_h bq[h] * Wk[h, c]
                pb = psum.tile([1, TAU], F32, tag="mm", name="pb")
                for nh in range(2):
                    nsl = slice(nh * 512, (nh + 1) * 512)
                    for j in range(HQ):
                        nc.tensor.matmul(
                            pb[:, nsl],
                            lhsT=bq_b[:, j : j + 1],
                            rhs=Wk_b[:, j, nsl],
                            start=(j == 0),
                            stop=(j == HQ - 1),
                        )
                wf_bias_b = wtmp.tile([128, TAU], BF16)
                nc.vector.memset(wf_bias_b, 0.0)
                nc.scalar.copy(out=wf_bias_b[0:1, :], in_=pb)
                ones_row_b = wtmp.tile([128, b_local], BF16)
                nc.vector.memset(ones_row_b, 0.0)
                nc.vector.memset(ones_row_b[0:1, :], 1.0)

                # qk[b, c] = msgs @ Wfused + qk_bias, scaled by 1/sqrt(H)
                for bi in range(n_btiles):
                    pq = psum.tile([128, TAU], F32, tag="mm")
                    for nh in range(2):
                        nsl = slice(nh * 512, (nh + 1) * 512)
                        for mi in range(MQ):
                            nc.tensor.matmul(
                                pq[:, nsl],
                                lhsT=msgsT_b[:, mi, bi * 128 : (bi + 1) * 128],
                                rhs=Wfused_b[:, mi, nsl],
                                start=(mi == 0),
                                stop=False,
                            )
                        nc.tensor.matmul(
                            pq[:, nsl],
                            lhsT=ones_row_b[:, bi * 128 : (bi + 1) * 128],
                            rhs=wf_bias_b[:, nsl],
                            start=False,
                            stop=True,
                        )
                    nc.scalar.mul(out=qk_b[bi], in_=pq, mul=1.0 / math.sqrt(HID))

            # ---------- main loop: stream tau, scores -> exp -> weighted sum ----
            with (
                tc.tile_pool(name="stream", bufs=chunk_bufs) as stream,
                tc.tile_pool(name="bfp", bufs=prod_bufs) as bfp,
                tc.tile_pool(name="aux", bufs=2) as aux,
                tc.tile_pool(name="outp", bufs=2) as outp,
            ):
                dump = aux.tile([128, TAU], BF16, tag="dump", name="dump", bufs=1)
                for bi in range(n_btiles):
                    bsl = slice(bi * 128, (bi + 1) * 128)
                    ctx_pp = [
                        aux.tile([128, TAU], BF16, tag="ctxA", name="ctxA"),
                        aux.tile([128, TAU], BF16, tag="ctxB", name="ctxB"),
                    ]
                    pp = 0
                    nc.vector.memset(ctx_pp[0], 0.0)
                    scores = aux.tile([128, T], F32, tag="scores", name="scores")
                    p_t = aux.tile([128, T], F32, tag="p", name="p_t")

                    for ci in range(n_chunks):
                        chunk_bf = stream.tile([128, chunk_free], BF16, tag="chunk", name="chunk_bf")
                        c0 = ci * chunk_free
                        if dma_cast:
                            nc.gpsimd.dma_start(
                                out=chunk_bf, in_=traj[bsl, c0 : c0 + chunk_free]
                            )
                        else:
                            chunk = bfp.tile([128, chunk_free], F32, tag="cf32", name="chunk")
                            nc.sync.dma_start(
                                out=chunk, in_=traj[bsl, c0 : c0 + chunk_free]
                            )
                            nc.scalar.copy(out=chunk_bf, in_=chunk)
                        prod = bfp.tile([128, chunk_free], BF16, tag="prod", name="prod")
                        if bcast_mult:
                            qk_rep = bass.AP(
                                tensor=qk_b[bi].tensor,
                                offset=qk_b[bi].offset,
                                ap=[qk_b[bi].ap[0], [0, t_chunk], [1, TAU]],
                            )
                            nc.vector.tensor_tensor(
                                out=prod,
                                in0=chunk_bf,
                                in1=qk_rep,
                                op=Alu.mult,
                            )
                        else:
                            for tt in range(t_chunk):
                                seg = slice(tt * TAU, (tt + 1) * TAU)
                                nc.vector.tensor_tensor(
                                    out=prod[:, seg],
                                    in0=chunk_bf[:, seg],
                                    in1=qk_b[bi],
                                    op=Alu.mult,
                                )
                        for tt in range(t_chunk):
                            col = ci * t_chunk + tt
                            seg = slice(tt * TAU, (tt + 1) * TAU)
                            if sum_engine == "scalar":
                                nc.scalar.activation(
                                    out=dump,
                                    in_=prod[:, seg],
                                    func=Act.Copy,
                                    accum_out=scores[:, col : col + 1],
                                )
                            else:
                                nc.vector.tensor_reduce(
                                    out=scores[:, col : col + 1],
                                    in_=prod[:, seg],
                                    axis=mybir.AxisListType.X,
                                    op=Alu.add,
                                )
                        csl = slice(ci * t_chunk, (ci + 1) * t_chunk)
                        nc.scalar.activation(
                            out=p_t[:, csl], in_=scores[:, csl], func=Act.Exp
                        )
                        for tt in range(t_chunk):
                            col = ci * t_chunk + tt
                            seg = slice(tt * TAU, (tt + 1) * TAU)
                            if tt < ts_act_per_chunk:
                                nc.scalar.activation(
                                    out=prod[:, seg],
                                    in_=chunk_bf[:, seg],
                                    func=Act.Copy,
                                    scale=p_t[:, col : col + 1],
                                )
                            else:
                                nc.vector.tensor_scalar(
                                    out=prod[:, seg],
                                    in0=chunk_bf[:, seg],
                                    scalar1=p_t[:, col : col + 1],
                                    scalar2=None,
                                    op0=Alu.mult,
                                )
                            src_ctx, dst_ctx = ctx_pp[pp], ctx_pp[1 - pp]
                            pp = 1 - pp
                            nc.vector.tensor_tensor(
                                out=dst_ctx,
                                in0=src_ctx,
                                in1=prod[:, seg],
                                op=Alu.add,
                            )

                    # normalize, project: out = (ctx / sum p) @ Wv.T + bv
                    s_sum = aux.tile([128, 1], F32, tag="ssum", name="s_sum")
                    nc.vector.tensor_reduce(
                        out=s_sum, in_=p_t, axis=mybir.AxisListType.X, op=Alu.add
                    )
                    rinv = aux.tile([128, 1], F32, tag="rinv", name="rinv")
                    nc.vector.reciprocal(out=rinv, in_=s_sum)
                    ctxn_f = aux.tile([128, TAU], F32, tag="ctxn", name="ctxn_f")
                    nc.vector.tensor_scalar(
                        out=ctxn_f,
                        in0=ctx_pp[pp],
                        scalar1=rinv,
                        scalar2=None,
                        op0=Alu.mult,
                    )
                    ctxT_b = aux.tile([128, CQ, 128], BF16, tag="ctxT", name="ctxT_b")
                    for j in range(CQ):
                        ptb = psum_tr.tile([128, 128], F32, tag="tr", name="ptb")
                        nc.tensor.transpose(
                            ptb, ctxn_f[:, j * 128 : (j + 1) * 128], ident_f
                        )
                        nc.scalar.copy(out=ctxT_b[:, j, :], in_=ptb)
                    pm = psum.tile([128, VDIM], F32, tag="mm", name="pm")
                    for j in range(CQ):
                        nc.tensor.matmul(
                            pm,
                            lhsT=ctxT_b[:, j, :],
                            rhs=WvT_b[:, j, :],
                            start=(j == 0),
                            stop=False,
                        )
                    nc.tensor.matmul(
                        pm,
                        lhsT=onespad_b,
                        rhs=bvpad_b,
                        start=False,
                        stop=True,
                    )
                    msg_out = outp.tile([128, VDIM], F32, tag="msg", name="msg_out")
                    nc.scalar.copy(out=msg_out, in_=pm)
                    nc.sync.dma_start(out=out[bsl, :], in_=msg_out)

    nc.compile()
    return nc


_NC_CACHE = {}


def _get_nc():
    key = "default"
    if key not in _NC_CACHE:
        _NC_CACHE[key] = build()
    return _NC_CACHE[key]


def make_in_maps(imagined_trajectory, received_messages, Wq, bq, Wk, Wv, bv):
    bl = B_LOCAL
    in_maps = []
    for i in range(N_CORES):
        sl = slice(i * bl, (i + 1) * bl)
        in_maps.append(
            {
                "imagined_trajectory": np.ascontiguousarray(
                    imagined_trajectory[sl], dtype=np.float32
                ),
                "received_messages": np.ascontiguousarray(
                    received_messages[sl], dtype=np.float32
                ),
                "Wq": np.asarray(Wq, dtype=np.float32),
                "bq": np.asarray(bq, dtype=np.float32),
                "Wk": np.asarray(Wk, dtype=np.float32),
                "Wv": np.asarray(Wv, dtype=np.float32),
                "bv": np.asarray(bv, dtype=np.float32),
            }
        )
    return in_maps


def kernel(
    imagined_trajectory,
    received_messages,
    Wq,
    bq,
    Wk,
    bk,
    Wv,
    bv,
):
    nc = _get_nc()
    in_maps = make_in_maps(
        imagined_trajectory, received_messages, Wq, bq, Wk, Wv, bv
    )
    res = run_bass_kernel_spmd(nc, in_maps, list(range(N_CORES)))
    return np.concatenate([res.results[i]["out"] for i in range(N_CORES)], axis=0)
